# revision 38
# baseline (speedup 1.0000x reference)
"""Trainium2 Bass kernel v3 for windowed multi-head attention with relative
position bias (nn_Attention_44006234915573).

Structure per window (625 tokens, d=128, 4 heads of 32):
  qkv = x @ Wqkv^T (PE, bf16), q|k packed in ONE 3-psum-bank tile
  (q at cols 0:625, k at 640:1265). Scores computed transposed and
  head-PAIR packed: one 3-bank psum tile holds S^T for heads (2p, 2p+1)
  at cols [0:625] and [640:1265] -> ONE ACT exp per stage (ACT is the
  bottleneck engine: 10 x ~1.35us/window is the exp floor). Bias applied
  as ONE pair-packed DVE multiply vs exp(bias) tables (bf16 2x mode).
  AV rides a fused ones-column in V' producing softmax denominators Z as
  extra psum rows; Z -> batched DRAM-bounce grid (1 gather DMA) ->
  reciprocal on [125,20] -> scatter -> 4 stride-0 broadcast loads.
  Normalize mults on GPSIMD (otherwise idle; SBUF-only bf16).
  Out-projection contracts the merged head-major normalized outputs.
  Window tail (Z chain, normalize, projection) is deferred into the next
  window's stage pipeline so ACT never starves.

Engine budget per window (target): ACT ~13.3us (cap), DVE ~12.1us,
PE ~8us warm / ~13us cold (HAM), GPSIMD ~5.4us, Sync/DMA ~8us.
Data parallel over windows: 32 per core x 8 cores.
"""

import sys
import types
import contextlib
import ctypes
from contextlib import ExitStack

import numpy as np
import ml_dtypes

import bass_rust as _bass_rust
import concourse.bass as bass
import concourse.tile as tile
from concourse import mybir
from concourse.vector_clock import ScopedClock

BATCH = 256
D = 128
WS = 25
N = WS * WS  # 625
H = 4
DH = 32
SCALE = DH**-0.5
NCORES = 8
WPC = BATCH // NCORES  # 32
JC = 5  # column chunks of 125
PCH = N // JC  # 125
P2 = 2  # head pairs

BF16 = mybir.dt.bfloat16
F32 = mybir.dt.float32

# pair-packed psum column layout: head A at [0:625], head B at [640:1265]
# (bank-aligned start); matmul writes may not cross psum bank boundaries
HB = 640  # head-B column base in pair-packed psum / e / btab tiles
EW = HB + N  # 1265
ETW = 1280  # sbuf e/btab tile width (even, 16-aligned)
PSW = 1536  # pair psum tile: 3 banks of 512 f32
# NOTE: concurrent matmuls on different tile_position row-groups writing
# the same PSUM bank fault the device (NRT_EXEC_UNIT_UNRECOVERABLE 101)
# when a ScalarE read follows. So S keeps v2's per-half 2-bank psum tiles;
# only the SBUF-side e/btab tiles are pair-packed.
AVCH = ((0, 512), (512, 113))  # i-chunks for S / AV / qk / proj psum writes


# ---------------------------------------------------------------------------
# workaround: this container's walrus rejects >1 sem wait on the kernel-tail
# Drain. Split the waits one-per-Drain.
def _patched_drain_and_barrier(self, tick_clock, wait_clock):
    nc = self.nc
    drain_inst = nc.sync.drain()
    wait_clock.add_sem_waits(
        drain_inst.ins, ScopedClock({None: tick_clock.global_clock})
    )
    si = drain_inst.ins.sync_info
    waits = list(si.on_wait)
    if len(waits) > 1:
        drain_inst.ins.sync_info = type(si)(on_wait=[], on_update=[])
        id2sem = {h.num: h for h in self.sems.allocated().values()}
        for w in waits:
            d = nc.sync.drain()
            _bass_rust.wait_op(d.ins, id2sem[w.id], w.wait_value, "sem-ge", False)
    nc.all_engine_barrier()
    popped = nc._tile_sem_poison_stack.pop()
    assert popped is self._sem_poison
    nc.clear_and_free_semaphores(list(self.sems.allocated().values()))
    nc.all_engine_barrier()


tile.TileContext._drain_and_barrier = _patched_drain_and_barrier


def _split_multi_waits(nc):
    """This walrus build accepts at most ONE sem wait per instruction; Tile's
    wait assignment can attach several. Move extras onto preceding nops on the
    same engine."""
    scratch_bb = nc.cur_bb.bb if nc.cur_bb is not None else None
    for f in nc.m.functions:
        for bb in f.blocks:
            lst = bb.instructions
            i = 0
            while i < len(lst):
                inst = lst[i]
                si = getattr(inst, "sync_info", None)
                if si is None:
                    i += 1
                    continue
                waits = list(si.on_wait)
                if len(waits) <= 1:
                    i += 1
                    continue
                SyncInfo = type(si)
                inst.sync_info = SyncInfo(
                    on_wait=[waits[-1]], on_update=list(si.on_update)
                )
                eng = nc.engines[inst.engine]
                for w in waits[:-1]:
                    nop = eng.nop(nofuse=True).ins
                    nop.sync_info = SyncInfo(on_wait=[w], on_update=[])
                    # eng.nop() appended to the current bb; move it here
                    for blk in f.blocks:
                        l2 = blk.instructions
                        if l2 and l2[-1] is nop:
                            l2.pop()
                            break
                    else:
                        if scratch_bb is not None:
                            l2 = scratch_bb.instructions
                            if l2 and l2[-1] is nop:
                                l2.pop()
                    lst.insert(i, nop)
                    i += 1
                i += 1


# ---------------------------------------------------------------------------
# NTFF profiling hook (only exercised when trace=True): the RL image's antenv
# lacks axon_hooks; install the ctypes equivalent of trn_boot's hook.
def _install_ntff_hook():
    if "antenv.axon_hooks" in sys.modules:
        return
    so_path = "/opt/axon/libaxon_pjrt.so"
    try:
        lib = ctypes.CDLL(so_path)
    except OSError:
        return
    if not hasattr(lib, "axon_start_nrt_profile"):
        return
    lib.axon_start_nrt_profile.argtypes = [
        ctypes.POINTER(ctypes.c_int64),
        ctypes.c_size_t,
    ]
    lib.axon_start_nrt_profile.restype = ctypes.c_int64
    lib.axon_stop_nrt_profile.argtypes = [ctypes.c_char_p]
    lib.axon_stop_nrt_profile.restype = ctypes.c_int64

    @contextlib.contextmanager
    def _hook(output_dir, device_ids=None):
        import jax

        jax.devices()
        if device_ids:
            ids = (ctypes.c_int64 * len(device_ids))(*device_ids)
            rc = lib.axon_start_nrt_profile(ids, len(device_ids))
        else:
            rc = lib.axon_start_nrt_profile(None, 0)
        if rc != 0:
            raise RuntimeError(f"axon_start_nrt_profile rc={rc}")
        try:
            yield
        finally:
            n = lib.axon_stop_nrt_profile(str(output_dir).encode())
            print(f"profile: {n} file(s) -> {output_dir}", file=sys.stderr)

    mod = types.ModuleType("antenv.axon_hooks")
    mod._hook = _hook
    mod.set_axon_ntff_profile_hook = lambda h: setattr(mod, "_hook", h)
    mod.get_axon_ntff_profile_hook = lambda: mod._hook
    sys.modules["antenv.axon_hooks"] = mod
    import antenv

    antenv.axon_hooks = mod


# ---------------------------------------------------------------------------
def build_nc(wpc=WPC, stages=5, gps_norm=False, gps_bias=(),
             recip_t=2, r2_t=3, norm_t=4, proj_t=8, sim_safe=False,
             av_lag=2, qkv_t=5):
    nc = bass.Bass(target_bir_lowering=False, debug=False)

    x_d = nc.dram_tensor("x", [wpc, D, N], BF16, kind="ExternalInput")
    wqk_d = nc.dram_tensor("wqk", [D, 2 * D], BF16, kind="ExternalInput")
    wv_d = nc.dram_tensor("wv", [D, D], BF16, kind="ExternalInput")
    # per head-pair W_out^T block: rows {0:32, 64:96} hold the pair's two
    # heads' contraction rows, rows 32:64 are ZERO (they meet garbage rows
    # of the onorm tiles)
    wo_d = nc.dram_tensor("wo", [P2, 96, D], BF16, kind="ExternalInput")
    expb_d = nc.dram_tensor("expb", [P2, JC, PCH, ETW], BF16,
                            kind="ExternalInput")
    y_d = nc.dram_tensor("y", [wpc, D, N], F32, kind="ExternalOutput")
    # Z rows bounced through DRAM for the free->partition grid transpose
    # and the cross-partition broadcast (bf16)
    zs_d = nc.dram_tensor("zscratch", [4, 2 * P2, N], BF16)
    rzs_d = nc.dram_tensor("rzscratch", [4, 2 * P2, N], BF16)

    with tile.TileContext(nc) as tc, ExitStack() as ctx:
        persist = ctx.enter_context(tc.tile_pool(name="persist", bufs=1))
        xpool = ctx.enter_context(tc.tile_pool(name="xpool", bufs=2))
        qkpool = ctx.enter_context(tc.tile_pool(name="qkpool", bufs=2))
        epool = ctx.enter_context(tc.tile_pool(name="epool", bufs=8))
        opool = ctx.enter_context(tc.tile_pool(name="opool", bufs=4))
        rpool = ctx.enter_context(tc.tile_pool(name="rpool", bufs=4))
        zpool = ctx.enter_context(tc.tile_pool(name="zpool", bufs=2))
        ypool = ctx.enter_context(tc.tile_pool(name="ypool", bufs=2))
        # PSUM: bigps 3x2 banks + avps 1x2 banks = 8 banks
        bigps = ctx.enter_context(tc.tile_pool(name="bigps", bufs=3, space="PSUM"))
        avps = ctx.enter_context(tc.tile_pool(name="avps", bufs=1, space="PSUM"))

        # --- persistent loads ------------------------------------------------
        wqk_sb = persist.tile([D, 2 * D], BF16, tag="wqk")
        nc.sync.dma_start(wqk_sb[:, :], wqk_d[:, :])
        wv_sb = persist.tile([D, D], BF16, tag="wv")
        nc.sync.dma_start(wv_sb[:, :], wv_d[:, :])
        wo_sb = []
        for p in range(P2):
            t = persist.tile([96, D], BF16, tag=f"wo{p}")
            nc.sync.dma_start(t[:, :], wo_d[p, :, :])
            wo_sb.append(t)

        btab = {}
        for p in range(P2):
            for jc in range(JC):
                t = persist.tile([PCH, ETW], BF16,
                                 name=f"btab{p}_{jc}", tag=f"btab{p}_{jc}")
                nc.sync.dma_start(t[:, :], expb_d[p, jc, :, :])
                btab[(p, jc)] = t

        # V' (n-major V with fused ones columns), double-buffered by window
        # parity to decouple window b+1's V eviction from window b's AV reads
        vprimes = []
        for v in range(2):
            t = persist.tile([PCH, JC * H * (DH + 1)], BF16, tag=f"vprime{v}")
            nc.vector.memset(t[:, :], 1.0)  # ones columns persist
            vprimes.append(t)

        # persistent per-(parity, pk) normalized-output tiles; rows {0:32,
        # 64:96} are written each window, rows 32:64 stay zero so the proj
        # matmul can contract all 96 rows against the zero-padded wo block
        onorms = [[persist.tile([96, N], BF16, name=f"on{v}_{p}",
                                tag=f"on{v}_{p}")
                   for p in range(P2)] for v in range(2)]
        for v in range(2):
            for p in range(P2):
                nc.vector.memset(onorms[v][p][32:64, :], 0.0)

        def vp(vt, jc, h):
            o = jc * H * (DH + 1) + h * (DH + 1)
            return vt[:, o : o + DH + 1]

        # --- per-window pipeline ---------------------------------------------
        xtiles = {}

        def load_x(b):
            if b < wpc and b not in xtiles:
                t = xpool.tile([D, N], BF16, tag="xb")
                nc.sync.dma_start(t[:, :], x_d[b, :, :])
                xtiles[b] = t

        # deferred work (window b-1 tails + window b+1 prologue), keyed by
        # due-stage in the current window
        tails = []

        def run_due(t):
            for item in list(tails):
                if item[0] <= t:
                    tails.remove(item)
                    item[1]()

        qks = {}

        def prologue_part(b, part):
            # one qkv part for window b: 2 MMs + a SCALAR-engine eviction
            # (ACT has idle headroom; keeps the DVE mult stream unblocked)
            xb = xtiles[b]
            base = part * HB
            ps = bigps.tile([D, 1024], F32, tag="big")
            for off, ln in AVCH:
                nc.tensor.matmul(
                    ps[:, off : off + ln],
                    lhsT=wqk_sb[:, part * D : (part + 1) * D],
                    rhs=xb[:, off : off + ln],
                    start=True,
                    stop=True,
                )
            nc.scalar.copy(qks[b][:, base : base + N], ps[:, :N])

        def prologue_v(b):
            xb = xtiles.pop(b)
            vprime = vprimes[b % 2]
            ps = bigps.tile([D, 1024], F32, tag="big")
            for jc in range(JC):
                nc.tensor.matmul(
                    ps[:PCH, jc * D : (jc + 1) * D],
                    lhsT=xb[:, jc * PCH : (jc + 1) * PCH],
                    rhs=wv_sb[:, :],
                    start=True,
                    stop=True,
                )
            vdst = vprime[:, :].rearrange(
                "p (j g c) -> p j g c", j=JC, g=H
            )[:, :, :, 0:DH]
            vsrc = ps[:PCH, : JC * D].rearrange("p (j g c) -> p j g c", j=JC, g=H)
            nc.vector.tensor_copy(vdst, vsrc)

        def prologue(b):
            # qkv + V for window b; hoisted into window b-1's stages so the
            # S/exp pipeline never drains at the window boundary
            if b >= wpc:
                return
            load_x(b + 1)
            qks[b] = qkpool.tile([D, ETW], BF16, tag="qk", name="qk")
            prologue_part(b, 0)
            prologue_part(b, 1)
            prologue_v(b)

        load_x(0)
        prologue(0)

        for b in range(wpc):
            qk = qks.pop(b)
            vprime = vprimes[b % 2]
            onorm = onorms[b % 2]

            if stages < 2:
                prologue(b + 1)
                ysb = ypool.tile([D, N], F32, tag="ysb")
                nc.vector.tensor_copy(ysb[:, :], qk[:, :N])
                nc.sync.dma_start(y_d[b, :, :], ysb[:, :])
                continue

            av = None
            osbs = {}
            stage_e = {}

            def emit_S(t):
                p, jc = divmod(t, JC)
                # per-half 2-bank psum tiles (see bank-conflict NOTE above);
                # both halves' exps land in ONE pair-packed sbuf e0 tile
                e0 = epool.tile([PCH, ETW], BF16, tag="e")
                for half in range(2):
                    h = 2 * p + half
                    sps = bigps.tile([D, 1024], F32, tag="big")
                    for off, ln in AVCH:
                        nc.tensor.matmul(
                            sps[:PCH, off : off + ln],
                            lhsT=qk[
                                DH * h : DH * (h + 1),
                                HB + jc * PCH : HB + (jc + 1) * PCH,
                            ],
                            rhs=qk[DH * h : DH * (h + 1), off : off + ln],
                            start=True,
                            stop=True,
                            tile_position=(DH * h, 0),
                        )
                    nc.scalar.activation(
                        e0[:, half * HB : half * HB + N], sps[:PCH, :N],
                        mybir.ActivationFunctionType.Exp,
                    )
                if stages == 2:
                    stage_e[t] = e0
                    return
                # ONE pair-packed bias multiply (DVE 2x bf16 mode) over the
                # two 625-col segments (skips the junk gap cols 625:640)
                e = epool.tile([PCH, ETW], BF16, tag="e")

                def seg(tile_):
                    v = tile_[:, :].rearrange("p (s c) -> p s c", s=2)
                    return v[:, :, 0:N]

                eng = nc.gpsimd if t in gps_bias else nc.vector
                eng.tensor_mul(seg(e), seg(e0), seg(btab[(p, jc)]))
                stage_e[t] = e

            def emit_AV_stub(t):
                es = stage_e.pop(t)
                if t == 2 * JC - 1:
                    ysb = ypool.tile([D, N], F32, tag="ysb")
                    nc.vector.tensor_copy(ysb[:PCH, :], es[:, :N])
                    nc.sync.dma_start(y_d[b, :, :], ysb[:, :])

            def emit_AV(t):
                nonlocal av
                p, jc = divmod(t, JC)
                if jc == 0:
                    av = avps.tile([D, 640], F32, tag="av")
                es = stage_e.pop(t)
                for off, ln in AVCH:
                    for half, rowbase in ((0, 0), (1, 64)):
                        h = 2 * p + half
                        base = half * HB
                        nc.tensor.matmul(
                            av[rowbase : rowbase + DH + 1, off : off + ln],
                            lhsT=vp(vprime, jc, h),
                            rhs=es[:, base + off : base + off + ln],
                            start=(jc == 0),
                            stop=(jc == JC - 1),
                            tile_position=(0, rowbase),
                            skip_group_check=True,
                        )
                if jc == JC - 1:
                    finish_pair(p)

            def finish_pair(pk):
                # O' + Z rows out of PSUM in bf16 (releases av for next pair;
                # bf16 enables DVE 2x + GPSIMD for the normalize mults)
                osb = opool.tile([D, N], BF16, tag="osb")
                if sim_safe:
                    nc.vector.tensor_copy(osb[:33, :], av[:33, :N])
                    nc.vector.tensor_copy(osb[64:97, :], av[64:97, :N])
                else:
                    nc.vector.tensor_copy(osb[:97, :], av[:97, :N])
                osbs[pk] = osb
                if stages < 4:
                    if pk == 1:
                        ysb = ypool.tile([D, N], F32, tag="ysb")
                        nc.vector.tensor_copy(ysb[:97, :], osb[:97, :])
                        nc.sync.dma_start(y_d[b, :, :], ysb[:, :])
                    return
                # Z rows (partitions 32 & 96) -> DRAM rows 2*pk, 2*pk+1
                zd = zs_d[b % 4]
                nc.sync.dma_start(zd[2 * pk, :], osb[32:33, :])
                nc.sync.dma_start(zd[2 * pk + 1, :], osb[96:97, :])
                if pk == 1 and stages >= 5:
                    schedule_tail(b, osbs, onorm)
                elif pk == 1:
                    # undeferred tail (debug)
                    fns = make_tail(b, osbs, onorm)
                    for _, fn in fns:
                        fn()

            def make_tail(b_, osbs_, onorm_):
                zd = zs_d[b_ % 4]
                rzd = rzs_d[b_ % 4]
                zg = zpool.tile([D, 32], BF16, tag="zg")
                rgb = zpool.tile([D, 32], BF16, tag="rgb")
                r2s = {}

                def t_gather():
                    # one DMA: dram [4, 625] -> grid [125, (z, 5)]
                    zap = zd[0, :]
                    src = bass.AP(zap.tensor, zap.offset,
                                  [[5, PCH], [N, 2 * P2], [1, 5]])
                    nc.sync.dma_start(zg[:PCH, 0:20], src)

                def t_recip():
                    # ONE cheap reciprocal on the [125, 20] grid (bf16 src,
                    # f32 out), convert back to bf16, scatter to DRAM
                    rg = zpool.tile([D, 32], F32, tag="rg")
                    nc.vector.reciprocal(rg[:PCH, 0:20], zg[:PCH, 0:20])
                    nc.vector.tensor_copy(rgb[:PCH, 0:20], rg[:PCH, 0:20])
                    rap = rzd[0, :]
                    dst = bass.AP(rap.tensor, rap.offset,
                                  [[5, PCH], [N, 2 * P2], [1, 5]])
                    nc.sync.dma_start(dst, rgb[:PCH, 0:20])

                def t_r2():
                    for pk in range(P2):
                        for a, r0 in ((0, 0), (1, 64)):
                            rap = rzd[2 * pk + a, :]
                            bsrc = bass.AP(rap.tensor, rap.offset,
                                           [[0, DH], [1, N]])
                            r2 = rpool.tile([D, N], BF16, tag=f"r2_{pk}_{a}")
                            nc.sync.dma_start(r2[r0 : r0 + DH, :], bsrc)
                            r2s[(pk, a)] = r2

                def t_norm(pk, a):
                    def fn():
                        r0 = 64 * a
                        eng = nc.gpsimd if gps_norm else nc.vector
                        eng.tensor_mul(
                            onorm_[pk][r0 : r0 + DH, :],
                            osbs_[pk][r0 : r0 + DH, :],
                            r2s[(pk, a)][r0 : r0 + DH, :],
                        )
                    return fn

                def t_proj():
                    pps = bigps.tile([D, 1024], F32, tag="big")
                    for off, ln in AVCH:
                        for p in range(P2):
                            nc.tensor.matmul(
                                pps[:, off : off + ln],
                                lhsT=wo_sb[p][:, :],
                                rhs=onorm_[p][:, off : off + ln],
                                start=(p == 0),
                                stop=(p == P2 - 1),
                            )
                    ysb = ypool.tile([D, N], F32, tag="ysb")
                    nc.vector.tensor_copy(ysb[:, :], pps[:, :N])
                    nc.sync.dma_start(y_d[b_, :, :], ysb[:, :])

                fns = [(0, t_gather), (recip_t, t_recip), (r2_t, t_r2)]
                for i, (pk, a) in enumerate(
                    ((0, 0), (0, 1), (1, 0), (1, 1))
                ):
                    fns.append((norm_t + i, t_norm(pk, a)))
                fns.append((proj_t, t_proj))
                return fns

            def schedule_tail(b_, osbs_, onorm_):
                tails.extend(make_tail(b_, osbs_, onorm_))

            av_fn = emit_AV if stages >= 3 else emit_AV_stub
            if b + 1 < wpc:
                def sched_prologue(b_):
                    load_x(b_ + 1)
                    qks[b_] = qkpool.tile([D, ETW], BF16, tag="qk",
                                          name="qk")
                    tails.append((qkv_t, lambda: prologue_part(b_, 0)))
                    tails.append((qkv_t + 1, lambda: prologue_part(b_, 1)))
                    tails.append((qkv_t + 2, lambda: prologue_v(b_)))
                sched_prologue(b + 1)
            for t in range(2 * JC):
                emit_S(t)
                if t >= av_lag:
                    av_fn(t - av_lag)
                run_due(t)  # deferred tail of b-1 + prologue of b+1
            for t in range(2 * JC - av_lag, 2 * JC):
                av_fn(t)

        # epilogue: last window's tail
        run_due(1000)
        for item in list(tails):
            item[1]()

    _split_multi_waits(nc)
    return nc


# ---------------------------------------------------------------------------
def host_prep(x, W_qkv, W_out, bias_table, rel_pos_indices):
    """Precompute the replicated device inputs (numpy, bf16)."""
    x = np.asarray(x, np.float32)
    W_qkv = np.asarray(W_qkv, np.float32)
    W_out = np.asarray(W_out, np.float32)
    bias_table = np.asarray(bias_table, np.float32)
    idx = np.asarray(rel_pos_indices)

    bf = ml_dtypes.bfloat16
    xb = x.reshape(BATCH, D, N).astype(bf)

    Wq = W_qkv[0:D] * SCALE
    Wk = W_qkv[D : 2 * D]
    Wv = W_qkv[2 * D : 3 * D]
    wqk = np.concatenate([Wq.T, Wk.T], axis=1).astype(bf)  # (128, 256)
    wv = Wv.T.astype(bf)  # (128, 128)
    WoT = W_out.T  # (c, dout), c head-major
    wo = np.zeros((P2, 96, D), np.float32)
    for p in range(P2):
        wo[p, 0:DH] = WoT[2 * p * DH : (2 * p + 1) * DH]
        wo[p, 64:96] = WoT[(2 * p + 1) * DH : (2 * p + 2) * DH]
    wo = wo.astype(bf)

    # bias^T per head: biast[h, j, i] = bias_table[idx[i, j], h]
    bfull = bias_table[idx]  # (i, j, H)
    biast = np.ascontiguousarray(np.transpose(bfull, (2, 1, 0)))  # (H, j, i)
    expb = np.exp(biast)  # (H, N, N)
    # pair-packed: head 2p at cols 0:625, head 2p+1 at cols 640:1265,
    # filler cols = 1.0 (junk-safe for the pair-packed multiply)
    expbp = np.ones((P2, JC, PCH, ETW), np.float32)
    for p in range(P2):
        for jc in range(JC):
            sl = slice(jc * PCH, (jc + 1) * PCH)
            expbp[p, jc, :, 0:N] = expb[2 * p, sl, :]
            expbp[p, jc, :, HB : HB + N] = expb[2 * p + 1, sl, :]
    return {
        "x": xb,
        "wqk": wqk,
        "wv": wv,
        "wo": wo,
        "expb": expbp.astype(bf),
    }


_NC_CACHE = {}


def _get_nc(wpc):
    import os

    stages = int(os.environ.get("K3_STAGES", "5"))
    gps_norm = bool(int(os.environ.get("K3_GPS_NORM", "0")))
    gps_bias = tuple(
        int(s) for s in os.environ.get("K3_GPS_BIAS", "").split(",") if s
    )
    recip_t = int(os.environ.get("K3_RECIP_T", "2"))
    r2_t = int(os.environ.get("K3_R2_T", "3"))
    norm_t = int(os.environ.get("K3_NORM_T", "3"))
    proj_t = int(os.environ.get("K3_PROJ_T", "8"))
    sim_safe = bool(int(os.environ.get("K3_SIM_SAFE", "0")))
    av_lag = int(os.environ.get("K3_AV_LAG", "2"))
    qkv_t = int(os.environ.get("K3_QKV_T", "5"))
    key = (wpc, stages, gps_norm, gps_bias, recip_t, r2_t, norm_t, proj_t,
           sim_safe, av_lag, qkv_t)
    if key not in _NC_CACHE:
        _NC_CACHE[key] = build_nc(
            wpc, stages=stages, gps_norm=gps_norm, gps_bias=gps_bias,
            recip_t=recip_t, r2_t=r2_t, norm_t=norm_t, proj_t=proj_t,
            sim_safe=sim_safe, av_lag=av_lag, qkv_t=qkv_t,
        )
    return _NC_CACHE[key]


def run(inputs, trace=False, wpc=WPC):
    """Run on 8 NeuronCores; returns (out, BassKernelResults)."""
    from concourse.bass_utils import run_bass_kernel_spmd

    if trace:
        _install_ntff_hook()
    prep = host_prep(
        inputs["x"], inputs["W_qkv"], inputs["W_out"],
        inputs["bias_table"], inputs["rel_pos_indices"],
    )
    shared = {k: v for k, v in prep.items() if k != "x"}
    xb = prep["x"]
    in_maps = [
        {"x": xb[i * wpc : (i + 1) * wpc], **shared} for i in range(NCORES)
    ]
    nc = _get_nc(wpc)
    res = run_bass_kernel_spmd(nc, in_maps, list(range(NCORES)), trace=trace)
    out = np.concatenate([res.results[i]["y"] for i in range(NCORES)], axis=0)
    out = out.reshape(BATCH, D, WS, WS).astype(np.float32)
    return out, res


def kernel(x, W_qkv, W_out, bias_table, rel_pos_indices):
    out, _ = run(
        {
            "x": x,
            "W_qkv": W_qkv,
            "W_out": W_out,
            "bias_table": bias_table,
            "rel_pos_indices": rel_pos_indices,
        },
        trace=False,
    )
    return out


# revision 40
# speedup vs baseline: 1.0140x; 1.0140x over previous
"""Trainium2 Bass kernel v3 for windowed multi-head attention with relative
position bias (nn_Attention_44006234915573).

Structure per window (625 tokens, d=128, 4 heads of 32):
  qkv = x @ Wqkv^T (PE, bf16), q|k packed in ONE 3-psum-bank tile
  (q at cols 0:625, k at 640:1265). Scores computed transposed and
  head-PAIR packed: one 3-bank psum tile holds S^T for heads (2p, 2p+1)
  at cols [0:625] and [640:1265] -> ONE ACT exp per stage (ACT is the
  bottleneck engine: 10 x ~1.35us/window is the exp floor). Bias applied
  as ONE pair-packed DVE multiply vs exp(bias) tables (bf16 2x mode).
  AV rides a fused ones-column in V' producing softmax denominators Z as
  extra psum rows; Z -> batched DRAM-bounce grid (1 gather DMA) ->
  reciprocal on [125,20] -> scatter -> 4 stride-0 broadcast loads.
  Normalize mults on GPSIMD (otherwise idle; SBUF-only bf16).
  Out-projection contracts the merged head-major normalized outputs.
  Window tail (Z chain, normalize, projection) is deferred into the next
  window's stage pipeline so ACT never starves.

Engine budget per window (target): ACT ~13.3us (cap), DVE ~12.1us,
PE ~8us warm / ~13us cold (HAM), GPSIMD ~5.4us, Sync/DMA ~8us.
Data parallel over windows: 32 per core x 8 cores.
"""

import sys
import types
import contextlib
import ctypes
from contextlib import ExitStack

import numpy as np
import ml_dtypes

import bass_rust as _bass_rust
import concourse.bass as bass
import concourse.tile as tile
from concourse import mybir
from concourse.vector_clock import ScopedClock

BATCH = 256
D = 128
WS = 25
N = WS * WS  # 625
H = 4
DH = 32
SCALE = DH**-0.5
NCORES = 8
WPC = BATCH // NCORES  # 32
JC = 5  # column chunks of 125
PCH = N // JC  # 125
P2 = 2  # head pairs

BF16 = mybir.dt.bfloat16
F32 = mybir.dt.float32

# pair-packed psum column layout: head A at [0:625], head B at [640:1265]
# (bank-aligned start); matmul writes may not cross psum bank boundaries
HB = 640  # head-B column base in pair-packed psum / e / btab tiles
EW = HB + N  # 1265
ETW = 1280  # sbuf e/btab tile width (even, 16-aligned)
PSW = 1536  # pair psum tile: 3 banks of 512 f32
# NOTE: concurrent matmuls on different tile_position row-groups writing
# the same PSUM bank fault the device (NRT_EXEC_UNIT_UNRECOVERABLE 101)
# when a ScalarE read follows. So S keeps v2's per-half 2-bank psum tiles;
# only the SBUF-side e/btab tiles are pair-packed.
AVCH = ((0, 512), (512, 113))  # i-chunks for S / AV / qk / proj psum writes


# ---------------------------------------------------------------------------
# workaround: this container's walrus rejects >1 sem wait on the kernel-tail
# Drain. Split the waits one-per-Drain.
def _patched_drain_and_barrier(self, tick_clock, wait_clock):
    nc = self.nc
    drain_inst = nc.sync.drain()
    wait_clock.add_sem_waits(
        drain_inst.ins, ScopedClock({None: tick_clock.global_clock})
    )
    si = drain_inst.ins.sync_info
    waits = list(si.on_wait)
    if len(waits) > 1:
        drain_inst.ins.sync_info = type(si)(on_wait=[], on_update=[])
        id2sem = {h.num: h for h in self.sems.allocated().values()}
        for w in waits:
            d = nc.sync.drain()
            _bass_rust.wait_op(d.ins, id2sem[w.id], w.wait_value, "sem-ge", False)
    nc.all_engine_barrier()
    popped = nc._tile_sem_poison_stack.pop()
    assert popped is self._sem_poison
    nc.clear_and_free_semaphores(list(self.sems.allocated().values()))
    nc.all_engine_barrier()


tile.TileContext._drain_and_barrier = _patched_drain_and_barrier


def _split_multi_waits(nc):
    """This walrus build accepts at most ONE sem wait per instruction; Tile's
    wait assignment can attach several. Move extras onto preceding nops on the
    same engine."""
    scratch_bb = nc.cur_bb.bb if nc.cur_bb is not None else None
    for f in nc.m.functions:
        for bb in f.blocks:
            lst = bb.instructions
            i = 0
            while i < len(lst):
                inst = lst[i]
                si = getattr(inst, "sync_info", None)
                if si is None:
                    i += 1
                    continue
                waits = list(si.on_wait)
                if len(waits) <= 1:
                    i += 1
                    continue
                SyncInfo = type(si)
                inst.sync_info = SyncInfo(
                    on_wait=[waits[-1]], on_update=list(si.on_update)
                )
                eng = nc.engines[inst.engine]
                for w in waits[:-1]:
                    nop = eng.nop(nofuse=True).ins
                    nop.sync_info = SyncInfo(on_wait=[w], on_update=[])
                    # eng.nop() appended to the current bb; move it here
                    for blk in f.blocks:
                        l2 = blk.instructions
                        if l2 and l2[-1] is nop:
                            l2.pop()
                            break
                    else:
                        if scratch_bb is not None:
                            l2 = scratch_bb.instructions
                            if l2 and l2[-1] is nop:
                                l2.pop()
                    lst.insert(i, nop)
                    i += 1
                i += 1


# ---------------------------------------------------------------------------
# NTFF profiling hook (only exercised when trace=True): the RL image's antenv
# lacks axon_hooks; install the ctypes equivalent of trn_boot's hook.
def _install_ntff_hook():
    if "antenv.axon_hooks" in sys.modules:
        return
    so_path = "/opt/axon/libaxon_pjrt.so"
    try:
        lib = ctypes.CDLL(so_path)
    except OSError:
        return
    if not hasattr(lib, "axon_start_nrt_profile"):
        return
    lib.axon_start_nrt_profile.argtypes = [
        ctypes.POINTER(ctypes.c_int64),
        ctypes.c_size_t,
    ]
    lib.axon_start_nrt_profile.restype = ctypes.c_int64
    lib.axon_stop_nrt_profile.argtypes = [ctypes.c_char_p]
    lib.axon_stop_nrt_profile.restype = ctypes.c_int64

    @contextlib.contextmanager
    def _hook(output_dir, device_ids=None):
        import jax

        jax.devices()
        if device_ids:
            ids = (ctypes.c_int64 * len(device_ids))(*device_ids)
            rc = lib.axon_start_nrt_profile(ids, len(device_ids))
        else:
            rc = lib.axon_start_nrt_profile(None, 0)
        if rc != 0:
            raise RuntimeError(f"axon_start_nrt_profile rc={rc}")
        try:
            yield
        finally:
            n = lib.axon_stop_nrt_profile(str(output_dir).encode())
            print(f"profile: {n} file(s) -> {output_dir}", file=sys.stderr)

    mod = types.ModuleType("antenv.axon_hooks")
    mod._hook = _hook
    mod.set_axon_ntff_profile_hook = lambda h: setattr(mod, "_hook", h)
    mod.get_axon_ntff_profile_hook = lambda: mod._hook
    sys.modules["antenv.axon_hooks"] = mod
    import antenv

    antenv.axon_hooks = mod


# ---------------------------------------------------------------------------
def build_nc(wpc=WPC, stages=5, gps_norm=False, gps_bias=(),
             recip_t=2, r2_t=3, norm_t=4, proj_t=8, sim_safe=False,
             av_lag=2, qkv_t=5):
    nc = bass.Bass(target_bir_lowering=False, debug=False)

    x_d = nc.dram_tensor("x", [wpc, D, N], BF16, kind="ExternalInput")
    wqk_d = nc.dram_tensor("wqk", [D, 2 * D], BF16, kind="ExternalInput")
    wv_d = nc.dram_tensor("wv", [D, D], BF16, kind="ExternalInput")
    # per head-pair W_out^T block: rows {0:32, 64:96} hold the pair's two
    # heads' contraction rows, rows 32:64 are ZERO (they meet garbage rows
    # of the onorm tiles)
    wo_d = nc.dram_tensor("wo", [P2, 96, D], BF16, kind="ExternalInput")
    expb_d = nc.dram_tensor("expb", [P2, JC, PCH, ETW], BF16,
                            kind="ExternalInput")
    y_d = nc.dram_tensor("y", [wpc, D, N], F32, kind="ExternalOutput")
    # Z rows bounced through DRAM for the free->partition grid transpose
    # and the cross-partition broadcast (bf16)
    zs_d = nc.dram_tensor("zscratch", [4, 2 * P2, N], BF16)
    rzs_d = nc.dram_tensor("rzscratch", [4, 2 * P2, N], BF16)

    with tile.TileContext(nc) as tc, ExitStack() as ctx:
        persist = ctx.enter_context(tc.tile_pool(name="persist", bufs=1))
        xpool = ctx.enter_context(tc.tile_pool(name="xpool", bufs=2))
        qkpool = ctx.enter_context(tc.tile_pool(name="qkpool", bufs=2))
        epool = ctx.enter_context(tc.tile_pool(name="epool", bufs=8))
        opool = ctx.enter_context(tc.tile_pool(name="opool", bufs=4))
        rpool = ctx.enter_context(tc.tile_pool(name="rpool", bufs=4))
        zpool = ctx.enter_context(tc.tile_pool(name="zpool", bufs=2))
        ypool = ctx.enter_context(tc.tile_pool(name="ypool", bufs=2))
        # PSUM: bigps 3x2 banks + avps 1x2 banks = 8 banks
        bigps = ctx.enter_context(tc.tile_pool(name="bigps", bufs=3, space="PSUM"))
        avps = ctx.enter_context(tc.tile_pool(name="avps", bufs=1, space="PSUM"))

        # --- persistent loads ------------------------------------------------
        wqk_sb = persist.tile([D, 2 * D], BF16, tag="wqk")
        nc.sync.dma_start(wqk_sb[:, :], wqk_d[:, :])
        wv_sb = persist.tile([D, D], BF16, tag="wv")
        nc.sync.dma_start(wv_sb[:, :], wv_d[:, :])
        wo_sb = []
        for p in range(P2):
            t = persist.tile([96, D], BF16, tag=f"wo{p}")
            nc.sync.dma_start(t[:, :], wo_d[p, :, :])
            wo_sb.append(t)

        btab = {}
        for p in range(P2):
            for jc in range(JC):
                t = persist.tile([PCH, ETW], BF16,
                                 name=f"btab{p}_{jc}", tag=f"btab{p}_{jc}")
                nc.sync.dma_start(t[:, :], expb_d[p, jc, :, :])
                btab[(p, jc)] = t

        # V' (n-major V with fused ones columns), double-buffered by window
        # parity to decouple window b+1's V eviction from window b's AV reads
        vprimes = []
        for v in range(2):
            t = persist.tile([PCH, JC * H * (DH + 1)], BF16, tag=f"vprime{v}")
            nc.vector.memset(t[:, :], 1.0)  # ones columns persist
            vprimes.append(t)

        # persistent per-(parity, pk) normalized-output tiles; rows {0:32,
        # 64:96} are written each window, rows 32:64 stay zero so the proj
        # matmul can contract all 96 rows against the zero-padded wo block
        onorms = [[persist.tile([96, N], BF16, name=f"on{v}_{p}",
                                tag=f"on{v}_{p}")
                   for p in range(P2)] for v in range(2)]
        for v in range(2):
            for p in range(P2):
                nc.vector.memset(onorms[v][p][32:64, :], 0.0)

        def vp(vt, jc, h):
            o = jc * H * (DH + 1) + h * (DH + 1)
            return vt[:, o : o + DH + 1]

        # --- per-window pipeline ---------------------------------------------
        xtiles = {}

        def load_x(b):
            if b < wpc and b not in xtiles:
                t = xpool.tile([D, N], BF16, tag="xb")
                nc.sync.dma_start(t[:, :], x_d[b, :, :])
                xtiles[b] = t

        # deferred work (window b-1 tails + window b+1 prologue), keyed by
        # due-stage in the current window
        tails = []

        def run_due(t):
            for item in list(tails):
                if item[0] <= t:
                    tails.remove(item)
                    item[1]()

        qks = {}

        def prologue_part(b, part):
            # one qkv part for window b: 2 MMs + a SCALAR-engine eviction
            # (ACT has idle headroom; keeps the DVE mult stream unblocked)
            xb = xtiles[b]
            base = part * HB
            ps = bigps.tile([D, 1024], F32, tag="big")
            for off, ln in AVCH:
                nc.tensor.matmul(
                    ps[:, off : off + ln],
                    lhsT=wqk_sb[:, part * D : (part + 1) * D],
                    rhs=xb[:, off : off + ln],
                    start=True,
                    stop=True,
                )
            nc.scalar.copy(qks[b][:, base : base + N], ps[:, :N])

        def prologue_v(b):
            xb = xtiles.pop(b)
            vprime = vprimes[b % 2]
            ps = bigps.tile([D, 1024], F32, tag="big")
            for jc in range(JC):
                nc.tensor.matmul(
                    ps[:PCH, jc * D : (jc + 1) * D],
                    lhsT=xb[:, jc * PCH : (jc + 1) * PCH],
                    rhs=wv_sb[:, :],
                    start=True,
                    stop=True,
                )
            vdst = vprime[:, :].rearrange(
                "p (j g c) -> p j g c", j=JC, g=H
            )[:, :, :, 0:DH]
            vsrc = ps[:PCH, : JC * D].rearrange("p (j g c) -> p j g c", j=JC, g=H)
            nc.vector.tensor_copy(vdst, vsrc)

        def prologue(b):
            # qkv + V for window b; hoisted into window b-1's stages so the
            # S/exp pipeline never drains at the window boundary
            if b >= wpc:
                return
            load_x(b + 1)
            qks[b] = qkpool.tile([D, ETW], BF16, tag="qk", name="qk")
            prologue_part(b, 0)
            prologue_part(b, 1)
            prologue_v(b)

        load_x(0)
        prologue(0)

        for b in range(wpc):
            qk = qks.pop(b)
            vprime = vprimes[b % 2]
            onorm = onorms[b % 2]

            if stages < 2:
                prologue(b + 1)
                ysb = ypool.tile([D, N], F32, tag="ysb")
                nc.vector.tensor_copy(ysb[:, :], qk[:, :N])
                nc.sync.dma_start(y_d[b, :, :], ysb[:, :])
                continue

            av = None
            osbs = {}
            stage_e = {}

            def emit_S(t):
                p, jc = divmod(t, JC)
                # per-half 2-bank psum tiles (see bank-conflict NOTE above);
                # both halves' exps land in ONE pair-packed sbuf e0 tile.
                # Chunk order (A512, B512, A113, B113) lets each LDWEIGHTS
                # hide under the other half's in-flight matmul (row groups
                # differ), instead of stalling on its own half.
                e0 = epool.tile([PCH, ETW], BF16, tag="e")
                sps = {}
                for half in range(2):
                    sps[half] = bigps.tile([D, 1024], F32, name="sps",
                                           tag="big")
                for off, ln in AVCH:
                    for half in range(2):
                        h = 2 * p + half
                        nc.tensor.matmul(
                            sps[half][:PCH, off : off + ln],
                            lhsT=qk[
                                DH * h : DH * (h + 1),
                                HB + jc * PCH : HB + (jc + 1) * PCH,
                            ],
                            rhs=qk[DH * h : DH * (h + 1), off : off + ln],
                            start=True,
                            stop=True,
                            tile_position=(DH * h, 0),
                        )
                for half in range(2):
                    nc.scalar.activation(
                        e0[:, half * HB : half * HB + N],
                        sps[half][:PCH, :N],
                        mybir.ActivationFunctionType.Exp,
                    )
                if stages == 2:
                    stage_e[t] = e0
                    return
                # ONE pair-packed bias multiply (DVE 2x bf16 mode) over the
                # two 625-col segments (skips the junk gap cols 625:640)
                e = epool.tile([PCH, ETW], BF16, tag="e")

                def seg(tile_):
                    v = tile_[:, :].rearrange("p (s c) -> p s c", s=2)
                    return v[:, :, 0:N]

                eng = nc.gpsimd if t in gps_bias else nc.vector
                eng.tensor_mul(seg(e), seg(e0), seg(btab[(p, jc)]))
                stage_e[t] = e

            def emit_AV_stub(t):
                es = stage_e.pop(t)
                if t == 2 * JC - 1:
                    ysb = ypool.tile([D, N], F32, tag="ysb")
                    nc.vector.tensor_copy(ysb[:PCH, :], es[:, :N])
                    nc.sync.dma_start(y_d[b, :, :], ysb[:, :])

            def emit_AV(t):
                nonlocal av
                p, jc = divmod(t, JC)
                if jc == 0:
                    av = avps.tile([D, 640], F32, tag="av")
                es = stage_e.pop(t)
                for off, ln in AVCH:
                    for half, rowbase in ((0, 0), (1, 64)):
                        h = 2 * p + half
                        base = half * HB
                        nc.tensor.matmul(
                            av[rowbase : rowbase + DH + 1, off : off + ln],
                            lhsT=vp(vprime, jc, h),
                            rhs=es[:, base + off : base + off + ln],
                            start=(jc == 0),
                            stop=(jc == JC - 1),
                            tile_position=(0, rowbase),
                            skip_group_check=True,
                        )
                if jc == JC - 1:
                    finish_pair(p)

            def finish_pair(pk):
                # O' + Z rows out of PSUM in bf16 (releases av for next pair;
                # bf16 enables DVE 2x + GPSIMD for the normalize mults)
                osb = opool.tile([D, N], BF16, tag="osb")
                if sim_safe:
                    nc.vector.tensor_copy(osb[:33, :], av[:33, :N])
                    nc.vector.tensor_copy(osb[64:97, :], av[64:97, :N])
                else:
                    nc.vector.tensor_copy(osb[:97, :], av[:97, :N])
                osbs[pk] = osb
                if stages < 4:
                    if pk == 1:
                        ysb = ypool.tile([D, N], F32, tag="ysb")
                        nc.vector.tensor_copy(ysb[:97, :], osb[:97, :])
                        nc.sync.dma_start(y_d[b, :, :], ysb[:, :])
                    return
                # Z rows (partitions 32 & 96) -> DRAM rows 2*pk, 2*pk+1
                zd = zs_d[b % 4]
                nc.sync.dma_start(zd[2 * pk, :], osb[32:33, :])
                nc.sync.dma_start(zd[2 * pk + 1, :], osb[96:97, :])
                if pk == 1 and stages >= 5:
                    schedule_tail(b, osbs, onorm)
                elif pk == 1:
                    # undeferred tail (debug)
                    fns = make_tail(b, osbs, onorm)
                    for _, fn in fns:
                        fn()

            def make_tail(b_, osbs_, onorm_):
                zd = zs_d[b_ % 4]
                rzd = rzs_d[b_ % 4]
                zg = zpool.tile([D, 32], BF16, tag="zg")
                rgb = zpool.tile([D, 32], BF16, tag="rgb")
                r2s = {}

                def t_gather():
                    # one DMA: dram [4, 625] -> grid [125, (z, 5)]
                    zap = zd[0, :]
                    src = bass.AP(zap.tensor, zap.offset,
                                  [[5, PCH], [N, 2 * P2], [1, 5]])
                    nc.sync.dma_start(zg[:PCH, 0:20], src)

                def t_recip():
                    # ONE cheap reciprocal on the [125, 20] grid (bf16 src,
                    # f32 out), convert back to bf16, scatter to DRAM
                    rg = zpool.tile([D, 32], F32, tag="rg")
                    nc.vector.reciprocal(rg[:PCH, 0:20], zg[:PCH, 0:20])
                    nc.vector.tensor_copy(rgb[:PCH, 0:20], rg[:PCH, 0:20])
                    rap = rzd[0, :]
                    dst = bass.AP(rap.tensor, rap.offset,
                                  [[5, PCH], [N, 2 * P2], [1, 5]])
                    nc.sync.dma_start(dst, rgb[:PCH, 0:20])

                def t_r2():
                    for pk in range(P2):
                        for a, r0 in ((0, 0), (1, 64)):
                            rap = rzd[2 * pk + a, :]
                            bsrc = bass.AP(rap.tensor, rap.offset,
                                           [[0, DH], [1, N]])
                            r2 = rpool.tile([D, N], BF16, tag=f"r2_{pk}_{a}")
                            nc.sync.dma_start(r2[r0 : r0 + DH, :], bsrc)
                            r2s[(pk, a)] = r2

                def t_norm(pk, a):
                    def fn():
                        r0 = 64 * a
                        eng = nc.gpsimd if gps_norm else nc.vector
                        eng.tensor_mul(
                            onorm_[pk][r0 : r0 + DH, :],
                            osbs_[pk][r0 : r0 + DH, :],
                            r2s[(pk, a)][r0 : r0 + DH, :],
                        )
                    return fn

                def t_proj():
                    pps = bigps.tile([D, 1024], F32, tag="big")
                    for off, ln in AVCH:
                        for p in range(P2):
                            nc.tensor.matmul(
                                pps[:, off : off + ln],
                                lhsT=wo_sb[p][:, :],
                                rhs=onorm_[p][:, off : off + ln],
                                start=(p == 0),
                                stop=(p == P2 - 1),
                            )
                    ysb = ypool.tile([D, N], F32, tag="ysb")
                    nc.vector.tensor_copy(ysb[:, :], pps[:, :N])
                    nc.sync.dma_start(y_d[b_, :, :], ysb[:, :])

                fns = [(0, t_gather), (recip_t, t_recip), (r2_t, t_r2)]
                for i, (pk, a) in enumerate(
                    ((0, 0), (0, 1), (1, 0), (1, 1))
                ):
                    fns.append((norm_t + i, t_norm(pk, a)))
                fns.append((proj_t, t_proj))
                return fns

            def schedule_tail(b_, osbs_, onorm_):
                tails.extend(make_tail(b_, osbs_, onorm_))

            av_fn = emit_AV if stages >= 3 else emit_AV_stub
            if b + 1 < wpc:
                def sched_prologue(b_):
                    load_x(b_ + 1)
                    qks[b_] = qkpool.tile([D, ETW], BF16, tag="qk",
                                          name="qk")
                    tails.append((qkv_t, lambda: prologue_part(b_, 0)))
                    tails.append((qkv_t + 1, lambda: prologue_part(b_, 1)))
                    tails.append((qkv_t + 2, lambda: prologue_v(b_)))
                sched_prologue(b + 1)
            for t in range(2 * JC):
                emit_S(t)
                if t >= av_lag:
                    av_fn(t - av_lag)
                run_due(t)  # deferred tail of b-1 + prologue of b+1
            for t in range(2 * JC - av_lag, 2 * JC):
                av_fn(t)

        # epilogue: last window's tail
        run_due(1000)
        for item in list(tails):
            item[1]()

    _split_multi_waits(nc)
    return nc


# ---------------------------------------------------------------------------
def host_prep(x, W_qkv, W_out, bias_table, rel_pos_indices):
    """Precompute the replicated device inputs (numpy, bf16)."""
    x = np.asarray(x, np.float32)
    W_qkv = np.asarray(W_qkv, np.float32)
    W_out = np.asarray(W_out, np.float32)
    bias_table = np.asarray(bias_table, np.float32)
    idx = np.asarray(rel_pos_indices)

    bf = ml_dtypes.bfloat16
    xb = x.reshape(BATCH, D, N).astype(bf)

    Wq = W_qkv[0:D] * SCALE
    Wk = W_qkv[D : 2 * D]
    Wv = W_qkv[2 * D : 3 * D]
    wqk = np.concatenate([Wq.T, Wk.T], axis=1).astype(bf)  # (128, 256)
    wv = Wv.T.astype(bf)  # (128, 128)
    WoT = W_out.T  # (c, dout), c head-major
    wo = np.zeros((P2, 96, D), np.float32)
    for p in range(P2):
        wo[p, 0:DH] = WoT[2 * p * DH : (2 * p + 1) * DH]
        wo[p, 64:96] = WoT[(2 * p + 1) * DH : (2 * p + 2) * DH]
    wo = wo.astype(bf)

    # bias^T per head: biast[h, j, i] = bias_table[idx[i, j], h]
    bfull = bias_table[idx]  # (i, j, H)
    biast = np.ascontiguousarray(np.transpose(bfull, (2, 1, 0)))  # (H, j, i)
    expb = np.exp(biast)  # (H, N, N)
    # pair-packed: head 2p at cols 0:625, head 2p+1 at cols 640:1265,
    # filler cols = 1.0 (junk-safe for the pair-packed multiply)
    expbp = np.ones((P2, JC, PCH, ETW), np.float32)
    for p in range(P2):
        for jc in range(JC):
            sl = slice(jc * PCH, (jc + 1) * PCH)
            expbp[p, jc, :, 0:N] = expb[2 * p, sl, :]
            expbp[p, jc, :, HB : HB + N] = expb[2 * p + 1, sl, :]
    return {
        "x": xb,
        "wqk": wqk,
        "wv": wv,
        "wo": wo,
        "expb": expbp.astype(bf),
    }


_NC_CACHE = {}


def _get_nc(wpc):
    import os

    stages = int(os.environ.get("K3_STAGES", "5"))
    gps_norm = bool(int(os.environ.get("K3_GPS_NORM", "0")))
    gps_bias = tuple(
        int(s) for s in os.environ.get("K3_GPS_BIAS", "").split(",") if s
    )
    recip_t = int(os.environ.get("K3_RECIP_T", "2"))
    r2_t = int(os.environ.get("K3_R2_T", "3"))
    norm_t = int(os.environ.get("K3_NORM_T", "3"))
    proj_t = int(os.environ.get("K3_PROJ_T", "8"))
    sim_safe = bool(int(os.environ.get("K3_SIM_SAFE", "0")))
    av_lag = int(os.environ.get("K3_AV_LAG", "2"))
    qkv_t = int(os.environ.get("K3_QKV_T", "5"))
    key = (wpc, stages, gps_norm, gps_bias, recip_t, r2_t, norm_t, proj_t,
           sim_safe, av_lag, qkv_t)
    if key not in _NC_CACHE:
        _NC_CACHE[key] = build_nc(
            wpc, stages=stages, gps_norm=gps_norm, gps_bias=gps_bias,
            recip_t=recip_t, r2_t=r2_t, norm_t=norm_t, proj_t=proj_t,
            sim_safe=sim_safe, av_lag=av_lag, qkv_t=qkv_t,
        )
    return _NC_CACHE[key]


def run(inputs, trace=False, wpc=WPC):
    """Run on 8 NeuronCores; returns (out, BassKernelResults)."""
    from concourse.bass_utils import run_bass_kernel_spmd

    if trace:
        _install_ntff_hook()
    prep = host_prep(
        inputs["x"], inputs["W_qkv"], inputs["W_out"],
        inputs["bias_table"], inputs["rel_pos_indices"],
    )
    shared = {k: v for k, v in prep.items() if k != "x"}
    xb = prep["x"]
    in_maps = [
        {"x": xb[i * wpc : (i + 1) * wpc], **shared} for i in range(NCORES)
    ]
    nc = _get_nc(wpc)
    res = run_bass_kernel_spmd(nc, in_maps, list(range(NCORES)), trace=trace)
    out = np.concatenate([res.results[i]["y"] for i in range(NCORES)], axis=0)
    out = out.reshape(BATCH, D, WS, WS).astype(np.float32)
    return out, res


def kernel(x, W_qkv, W_out, bias_table, rel_pos_indices):
    out, _ = run(
        {
            "x": x,
            "W_qkv": W_qkv,
            "W_out": W_out,
            "bias_table": bias_table,
            "rel_pos_indices": rel_pos_indices,
        },
        trace=False,
    )
    return out
